# revision 21
# baseline (speedup 1.0000x reference)
"""Trainium2 Bass kernel for DepthSeparableConv2d (dw3x3 + BN + ReLU + max-abs
prune + pw1x1 + BN + ReLU + prune), batch-data-parallel over 8 NeuronCores.

Per-core program (4 batches, 2 channel blocks => 8 units):
  - x staged into a zero-padded [P,58,58] SBUF tile; padding makes every PE
    tap a CONTIGUOUS flat 464-elem matmul window (strided rhs splits into
    multiple instructions, measured 2.5x slower) and removes all edge cases
  - depthwise conv is exact fp32 end-to-end: the 4.0 prune threshold has
    plane margins down to 1.2e-4 absolute, so any reduced-precision conv
    (bf16/f32r) flips planes worth up to 0.31 rel err (measured on host)
  - fp32 matmul measured at 1 cyc/row on this hw (376ns per 448-row matmul,
    same as f32r), so PE takes most rows: NPE chunks of 8 rows per unit via
    diagonal matmuls into PSUM; the rest is a DVE span of in-place
    scalar_tensor_tensor MACs (1.15ns/elem measured, alignment-insensitive)
  - GpSimd does no compute (measured 14ns/elem for tensor_scalar): memsets
    and weight DMA only
  - BN1 s1 pre-folded into dw weights on host; t1 added at span init / chunk
    drain; yt = relu(acc) via ACT
  - mask: one ACT pass per unit over yt: sum(relu(yt-4)) > 0 <=> plane max>4
    (margin-checked); applied to pointwise weights. Optional per-unit switch
    to DVE tensor_reduce(max) for load balancing.
  - pointwise conv as fp32 matmuls (K=256 in 2 chunks), BN2+ReLU via ACT
    draining multi-bank PSUM groups, z stored as bf16 (host converts back;
    0.4% rounding << 2e-2 budget, z-prune is a numerical no-op)
"""
import os
import sys
if "/opt/trn_rl_repo" not in sys.path:
    sys.path.insert(0, "/opt/trn_rl_repo")
os.environ.setdefault("NEURON_RT_RESET_CORES", "1")

import numpy as np
import concourse.bacc as bacc
import concourse.tile as tile
from concourse import mybir
from concourse.bass_utils import run_bass_kernel_spmd

EPS = 1e-5
DW_THRESH = 4.0
NCORES = 8
B_PER = 4            # batches per core
C = 256              # input channels
O = 256              # output channels
H = W = 56
P = 128              # partitions
NCB = C // P         # channel blocks
NOB = O // P
NCH = 8              # rows per PE conv chunk
PW = W + 2           # padded width
PH = H + 2           # padded height

# per-unit tunables (unit = b*NCB+cb)
NPE_LIST = [3, 3, 2, 2, 2, 2, 2, 2]   # PE chunks of 8 rows per unit
INIT_ON_DVE = [False] * 8             # span init: DVE tensor_scalar vs ACT
MAX_SPAN = 40                         # >= max DVE span rows

TAPS = [(0, 0), (0, -1), (0, 1),
        (-1, 0), (-1, -1), (-1, 1),
        (1, 0), (1, -1), (1, 1)]

F32 = mybir.dt.float32
F32R = mybir.dt.float32r
BF16 = mybir.dt.bfloat16

# test-harness hooks (grader path leaves these untouched)
TRACE = False
LAST_RESULTS = None


def _install_trace_hook():
    import types
    import antenv
    if hasattr(antenv, "axon_hooks"):
        return
    _m = types.ModuleType("antenv.axon_hooks")
    _h = [None]
    _m.set_axon_ntff_profile_hook = lambda hook: _h.__setitem__(0, hook)
    _m.get_axon_ntff_profile_hook = lambda: _h[0]
    sys.modules["antenv.axon_hooks"] = _m
    antenv.axon_hooks = _m
    from trn_agent_boot.trn_boot import _ntff_profile_via_ctypes
    _m.set_axon_ntff_profile_hook(
        _ntff_profile_via_ctypes("/opt/axon/libaxon_pjrt.so"))


def _build():
    nc = bacc.Bacc("TRN2", target_bir_lowering=False, debug=False,
                   num_devices=NCORES)
    x = nc.dram_tensor("x", [B_PER, C, H, W], F32, kind="ExternalInput").ap()
    wtap = nc.dram_tensor("wtap", [NCB, 9, P], F32, kind="ExternalInput").ap()
    pwt = nc.dram_tensor("pwt", [NCB, P, O], F32, kind="ExternalInput").ap()
    vecs = nc.dram_tensor("vecs", [4, NCB, P], F32, kind="ExternalInput").ap()
    t2d = nc.dram_tensor("t2d", [NOB, P], F32, kind="ExternalInput").ap()
    zout = nc.dram_tensor("z", [B_PER, O, H, W], BF16, kind="ExternalOutput").ap()

    with tile.TileContext(nc) as tc:
        with tc.tile_pool(name="singles", bufs=1) as singles, \
             tc.tile_pool(name="xp", bufs=3) as xp, \
             tc.tile_pool(name="accd", bufs=2) as accdp, \
             tc.tile_pool(name="accb", bufs=2) as accbp, \
             tc.tile_pool(name="yp", bufs=5) as yp, \
             tc.tile_pool(name="zp", bufs=2) as zp, \
             tc.tile_pool(name="smallp", bufs=8) as smallp, \
             tc.tile_pool(name="wmp", bufs=8) as wmp, \
             tc.tile_pool(name="psc", bufs=2, space="PSUM") as psc, \
             tc.tile_pool(name="psw", bufs=2, space="PSUM") as psw:

            # ---- constants (small vectors first so x loads start early;
            # the diagonal weight matrices are built on device: 1.18MB of
            # mostly-zero DMA replaced by an identity mask + 18 tiny muls) ----
            vv = singles.tile([P, 4, NCB], F32, tag="vv")
            nc.sync.dma_start(out=vv, in_=vecs.rearrange("v c k -> k v c"))
            wt = singles.tile([P, NCB, 9], F32, tag="wt")
            nc.sync.dma_start(out=wt, in_=wtap.rearrange("c t k -> k c t"))
            dg = singles.tile([P, NCB, 9, P], F32, tag="dg")
            ident = singles.tile([P, P], F32, tag="ident")
            from concourse.masks import make_identity
            make_identity(nc, ident)
            for _cb in range(NCB):
                for _ti in range(9):
                    nc.vector.tensor_scalar_mul(
                        dg[:, _cb, _ti, :], ident, wt[:, _cb, _ti : _ti + 1])
            pw = singles.tile([P, NCB, O], F32, tag="pw")
            nc.gpsimd.dma_start(out=pw, in_=pwt.rearrange("c k o -> k c o"))
            t2v = singles.tile([P, NOB], F32, tag="t2v")
            nc.gpsimd.dma_start(out=t2v, in_=t2d.rearrange("c k -> k c"))
            # vecs rows: 0=t1, 1=s2, 2=(t1-4), 3=-4
            scratch = singles.tile([P, H * W], F32, tag="scratch")

            units = [(b, cb) for b in range(B_PER) for cb in range(NCB)]
            xts = {}

            def emit_dma_and_init(u):
                b, cb = units[u]
                t1 = vv[:, 0, cb : cb + 1]
                w00 = wt[:, cb, 0:1]
                xt = xp.tile([P, PH, PW], F32, tag="xt")
                if u < 3:
                    # first use of this pool buffer: zero the pad strips
                    # (DMA only ever writes the interior, so they stay zero)
                    xf = xt.rearrange("p a b -> p (a b)")
                    nc.gpsimd.memset(xf[:, 0:PW], 0.0)
                    nc.gpsimd.memset(xf[:, (PH - 1) * PW : PH * PW], 0.0)
                    nc.gpsimd.memset(xt[:, 1 : PH - 1, 0:1], 0.0)
                    nc.gpsimd.memset(xt[:, 1 : PH - 1, PW - 1 : PW], 0.0)
                for qi, (ra, rb) in enumerate(((0, 10), (10, 20), (20, 30),
                                               (30, 40), (40, 48), (48, 56))):
                    q = nc.sync if qi % 2 == 0 else nc.scalar
                    q.dma_start(
                        out=xt[:, 1 + ra : 1 + rb, 1 : 1 + W],
                        in_=x[b, cb * P : (cb + 1) * P, ra:rb])
                d0 = NPE_LIST[u] * NCH
                rows = H - d0
                ad = accdp.tile([P, MAX_SPAN, W], F32, tag="ad")
                ab = accbp.tile([P, MAX_SPAN, W], F32, tag="ab")
                src = xt[:, 1 + d0 : 1 + H, 1 : 1 + W]
                # two span accumulators: acc_a = x*(s1*w00) + t1 (tap (0,0)),
                # acc_b = x_shift*(s1*w7) (tap (1,0)); GpSimd merges them so
                # DVE runs 7 tap MACs instead of 8
                nc.scalar.activation(
                    out=ad[:, :rows, :], in_=src,
                    func=mybir.ActivationFunctionType.Identity,
                    scale=w00, bias=t1)
                dr7, dc7 = TAPS[7]
                nc.scalar.activation(
                    out=ab[:, :rows, :],
                    in_=xt[:, 1 + d0 + dr7 : 1 + H + dr7,
                           1 + dc7 : 1 + W + dc7],
                    func=mybir.ActivationFunctionType.Copy,
                    scale=wt[:, cb, 7:8], bias=0.0)
                xts[u] = (xt, ad, ab)

            def emit_conv(u):
                b, cb = units[u]
                xt, ad, ab = xts[u]
                t1 = vv[:, 0, cb : cb + 1]
                t1m4 = vv[:, 2, cb : cb + 1]
                npe = NPE_LIST[u]
                d0 = npe * NCH
                rows = H - d0
                # yt is bf16 (fastest matmul dtype, z tolerance 2e-2 vs
                # ~5e-3 realized); the mask partials read the fp32 PSUM/acc
                # sources, so prune decisions stay exact
                yt = yp.tile([P, H, W], BF16, tag="yt")
                ysum = smallp.tile([P, npe + 1], F32, tag="ysum")
                xf = xt.rearrange("p a b -> p (a b)")

                # --- DVE span: tap 8 onto acc_b, taps 1-6 onto acc_a;
                # GpSimd folds acc_b into acc_a (tensor_tensor add) ---
                for ti in (8, 1, 2, 3, 4, 5, 6):
                    dr, dc = TAPS[ti]
                    sc = wt[:, cb, ti : ti + 1]
                    acc = ab if ti == 8 else ad
                    nc.vector.scalar_tensor_tensor(
                        out=acc[:, :rows, :],
                        in0=xt[:, 1 + d0 + dr : 1 + H + dr, 1 + dc : 1 + W + dc],
                        scalar=sc, in1=acc[:, :rows, :],
                        op0=mybir.AluOpType.mult, op1=mybir.AluOpType.add)
                nc.gpsimd.tensor_tensor(
                    out=ad[:, :rows, :].rearrange("p h w -> p (h w)"),
                    in0=ad[:, :rows, :].rearrange("p h w -> p (h w)"),
                    in1=ab[:, :rows, :].rearrange("p h w -> p (h w)"),
                    op=mybir.AluOpType.add)

                # --- PE chunks: 9 contiguous flat-window matmuls each ---
                for ci in range(npe):
                    r0 = ci * NCH
                    pt = psc.tile([P, NCH, PW], F32, tag="pt")
                    po = pt.rearrange("p a b -> p (a b)")
                    for ti, (dr, dc) in enumerate(TAPS):
                        s0 = (r0 + 1 + dr) * PW + 1 + dc
                        nc.tensor.matmul(
                            po[:, : NCH * PW], dg[:, cb, ti, :],
                            xf[:, s0 : s0 + NCH * PW],
                            start=(ti == 0), stop=(ti == 8))
                    nc.scalar.activation(
                        out=yt[:, r0 : r0 + NCH, :].rearrange("p h w -> p (h w)"),
                        in_=pt[:, :, 0:W],
                        func=mybir.ActivationFunctionType.Relu,
                        scale=1.0, bias=t1)
                    nc.scalar.activation(
                        out=scratch[:, : NCH * W], in_=pt[:, :, 0:W],
                        func=mybir.ActivationFunctionType.Relu,
                        scale=1.0, bias=t1m4,
                        accum_out=ysum[:, ci : ci + 1])

                # --- yt + mask partial from the span (acc already has t1) ---
                av = ad[:, :rows, :].rearrange("p h w -> p (h w)")
                nc.scalar.activation(
                    out=yt[:, d0:H, :].rearrange("p h w -> p (h w)"),
                    in_=av, func=mybir.ActivationFunctionType.Relu)
                nc.scalar.activation(
                    out=scratch[:, : rows * W], in_=av,
                    func=mybir.ActivationFunctionType.Relu,
                    bias=vv[:, 3, 0:1],
                    accum_out=ysum[:, npe : npe + 1])
                return yt, ysum

            def emit_mask(u, ysum):
                # plane max(y) > 4 <=> sum(relu(y-4)) > 0 (margin-checked)
                tot = smallp.tile([P, 1], F32, tag="tot")
                nc.vector.tensor_reduce(
                    out=tot, in_=ysum, axis=mybir.AxisListType.X,
                    op=mybir.AluOpType.add)
                m1 = smallp.tile([P, 1], F32, tag="m1")
                nc.vector.tensor_scalar(
                    out=m1, in0=tot, scalar1=0.0, scalar2=None,
                    op0=mybir.AluOpType.is_gt)
                return m1

            def emit_pw(b, ys, m1s):
                masks = []
                for cb in range(NCB):
                    wm = wmp.tile([P, O], BF16, tag="wm")
                    nc.vector.tensor_scalar_mul(wm, pw[:, cb, :], m1s[cb])
                    masks.append(wm)
                for ob in range(NOB):
                    s2 = vv[:, 1, ob : ob + 1]
                    t2 = t2v[:, ob : ob + 1]
                    zt = zp.tile([P, H, W], BF16, tag="zt")
                    for g0, gn in ((0, 3), (3, 3), (6, 1)):
                        pz = psw.tile([P, 3, 512], F32, tag="pz")
                        for kb in range(NCB):
                            for gi in range(gn):
                                r0 = (g0 + gi) * NCH
                                rhs = ys[kb][:, r0 : r0 + NCH, :].rearrange(
                                    "p h w -> p (h w)")
                                nc.tensor.matmul(
                                    pz[:, gi, : NCH * W],
                                    masks[kb][:, ob * P : (ob + 1) * P], rhs,
                                    start=(kb == 0), stop=(kb == NCB - 1))
                        nc.scalar.activation(
                            out=zt[:, g0 * NCH : (g0 + gn) * NCH, :].rearrange(
                                "p h w -> p (h w)"),
                            in_=pz[:, :gn, : NCH * W],
                            func=mybir.ActivationFunctionType.Relu,
                            scale=s2, bias=t2)
                    for za, zb in [(k * 7, k * 7 + 7) for k in range(8)]:
                        nc.gpsimd.dma_start(
                            out=zout[b, ob * P : (ob + 1) * P, za:zb],
                            in_=zt[:, za:zb, :])

            # ---- software-pipelined emission ----
            # mask ops are pended one unit (so the in-order DVE/ACT queues
            # don't stall on yt completion ahead of the next unit's work);
            # PW for batch b lands right after batch b+1's first conv unit
            yts = {}
            ysums = {}
            m1s = {}
            emit_dma_and_init(0)
            for u in range(len(units)):
                if u + 1 < len(units):
                    emit_dma_and_init(u + 1)
                yts[u], ysums[u] = emit_conv(u)
                if u >= 2:
                    pu = u - 2
                    m1s[pu] = emit_mask(pu, ysums[pu])
                    pb, pcb = units[pu]
                    if pcb == NCB - 1:
                        emit_pw(pb, [yts[pb * NCB + k] for k in range(NCB)],
                                [m1s[pb * NCB + k] for k in range(NCB)])
            for pu in (len(units) - 2, len(units) - 1):
                m1s[pu] = emit_mask(pu, ysums[pu])
                pb, pcb = units[pu]
                if pcb == NCB - 1:
                    emit_pw(pb, [yts[pb * NCB + k] for k in range(NCB)],
                            [m1s[pb * NCB + k] for k in range(NCB)])

    nc.compile()
    return nc


def kernel(x, dw_w, dw_b, bn1_gamma, bn1_beta, bn1_mean, bn1_var,
           pw_w, pw_b, bn2_gamma, bn2_beta, bn2_mean, bn2_var):
    # ---- host-side parameter folding (O(C) work only) ----
    s1 = (bn1_gamma / np.sqrt(bn1_var + EPS)).astype(np.float32)
    t1 = ((dw_b - bn1_mean) * s1 + bn1_beta).astype(np.float32)
    s2 = (bn2_gamma / np.sqrt(bn2_var + EPS)).astype(np.float32)
    t2 = ((pw_b - bn2_mean) * s2 + bn2_beta).astype(np.float32)

    # dw weights pre-scaled by s1: conv output is pre-relu y minus t1
    wfold = (dw_w[:, 0, :, :] * s1[:, None, None]).astype(np.float32)  # [C,3,3]
    wtap = np.zeros((NCB, 9, P), dtype=np.float32)
    for cb in range(NCB):
        for ti, (dr, dc) in enumerate(TAPS):
            wtap[cb, ti] = wfold[cb * P : (cb + 1) * P, dr + 1, dc + 1]

    pwt = np.ascontiguousarray(
        pw_w[:, :, 0, 0].T.reshape(NCB, P, O)).astype(np.float32)
    vecs = np.stack([t1.reshape(NCB, P), s2.reshape(NCB, P),
                     (t1 - DW_THRESH).reshape(NCB, P),
                     np.full((NCB, P), -DW_THRESH, np.float32)], axis=0)
    t2d = t2.reshape(NOB, P)

    nc = _build()

    in_maps = []
    for c in range(NCORES):
        in_maps.append({
            "x": np.ascontiguousarray(x[c * B_PER : (c + 1) * B_PER]),
            "wtap": wtap, "pwt": pwt,
            "vecs": np.ascontiguousarray(vecs), "t2d": np.ascontiguousarray(t2d),
        })
    if TRACE:
        _install_trace_hook()
    res = run_bass_kernel_spmd(nc, in_maps, core_ids=list(range(NCORES)),
                               trace=TRACE)
    global LAST_RESULTS
    LAST_RESULTS = res
    out = np.concatenate(
        [res.results[c]["z"].astype(np.float32) for c in range(NCORES)], axis=0)
    return out


# revision 22
# speedup vs baseline: 1.1111x; 1.1111x over previous
"""Trainium2 Bass kernel for DepthSeparableConv2d (dw3x3 + BN + ReLU + max-abs
prune + pw1x1 + BN + ReLU + prune), batch-data-parallel over 8 NeuronCores.

Per-core program (4 batches, 2 channel blocks => 8 units):
  - x staged into a zero-padded [P,58,58] SBUF tile; padding makes every PE
    tap a CONTIGUOUS flat 464-elem matmul window (strided rhs splits into
    multiple instructions, measured 2.5x slower) and removes all edge cases
  - depthwise conv is exact fp32 end-to-end: the 4.0 prune threshold has
    plane margins down to 1.2e-4 absolute, so any reduced-precision conv
    (bf16/f32r) flips planes worth up to 0.31 rel err (measured on host)
  - fp32 matmul measured at 1 cyc/row on this hw (376ns per 448-row matmul,
    same as f32r), so PE takes most rows: NPE chunks of 8 rows per unit via
    diagonal matmuls into PSUM; the rest is a DVE span of in-place
    scalar_tensor_tensor MACs (1.15ns/elem measured, alignment-insensitive)
  - GpSimd does no compute (measured 14ns/elem for tensor_scalar): memsets
    and weight DMA only
  - BN1 s1 pre-folded into dw weights on host; t1 added at span init / chunk
    drain; yt = relu(acc) via ACT
  - mask: one ACT pass per unit over yt: sum(relu(yt-4)) > 0 <=> plane max>4
    (margin-checked); applied to pointwise weights. Optional per-unit switch
    to DVE tensor_reduce(max) for load balancing.
  - pointwise conv as fp32 matmuls (K=256 in 2 chunks), BN2+ReLU via ACT
    draining multi-bank PSUM groups, z stored as bf16 (host converts back;
    0.4% rounding << 2e-2 budget, z-prune is a numerical no-op)
"""
import os
import sys
if "/opt/trn_rl_repo" not in sys.path:
    sys.path.insert(0, "/opt/trn_rl_repo")
os.environ.setdefault("NEURON_RT_RESET_CORES", "1")

import numpy as np
import concourse.bacc as bacc
import concourse.tile as tile
from concourse import mybir
from concourse.bass_utils import run_bass_kernel_spmd

EPS = 1e-5
DW_THRESH = 4.0
NCORES = 8
B_PER = 4            # batches per core
C = 256              # input channels
O = 256              # output channels
H = W = 56
P = 128              # partitions
NCB = C // P         # channel blocks
NOB = O // P
NCH = 8              # rows per PE conv chunk
PW = W + 2           # padded width
PH = H + 2           # padded height

# per-unit tunables (unit = b*NCB+cb)
NPE_LIST = [3, 3, 3, 2, 2, 2, 2, 2]   # PE chunks of 8 rows per unit
INIT_ON_DVE = [False] * 8             # span init: DVE tensor_scalar vs ACT
MAX_SPAN = 40                         # >= max DVE span rows

TAPS = [(0, 0), (0, -1), (0, 1),
        (-1, 0), (-1, -1), (-1, 1),
        (1, 0), (1, -1), (1, 1)]

F32 = mybir.dt.float32
F32R = mybir.dt.float32r
BF16 = mybir.dt.bfloat16

# test-harness hooks (grader path leaves these untouched)
TRACE = False
LAST_RESULTS = None


def _install_trace_hook():
    import types
    import antenv
    if hasattr(antenv, "axon_hooks"):
        return
    _m = types.ModuleType("antenv.axon_hooks")
    _h = [None]
    _m.set_axon_ntff_profile_hook = lambda hook: _h.__setitem__(0, hook)
    _m.get_axon_ntff_profile_hook = lambda: _h[0]
    sys.modules["antenv.axon_hooks"] = _m
    antenv.axon_hooks = _m
    from trn_agent_boot.trn_boot import _ntff_profile_via_ctypes
    _m.set_axon_ntff_profile_hook(
        _ntff_profile_via_ctypes("/opt/axon/libaxon_pjrt.so"))


def _build():
    nc = bacc.Bacc("TRN2", target_bir_lowering=False, debug=False,
                   num_devices=NCORES)
    x = nc.dram_tensor("x", [B_PER, C, H, W], F32, kind="ExternalInput").ap()
    wtap = nc.dram_tensor("wtap", [NCB, 9, P], F32, kind="ExternalInput").ap()
    pwt = nc.dram_tensor("pwt", [NCB, P, O], F32, kind="ExternalInput").ap()
    vecs = nc.dram_tensor("vecs", [4, NCB, P], F32, kind="ExternalInput").ap()
    t2d = nc.dram_tensor("t2d", [NOB, P], F32, kind="ExternalInput").ap()
    zout = nc.dram_tensor("z", [B_PER, O, H, W], BF16, kind="ExternalOutput").ap()

    with tile.TileContext(nc) as tc:
        with tc.tile_pool(name="singles", bufs=1) as singles, \
             tc.tile_pool(name="xp", bufs=3) as xp, \
             tc.tile_pool(name="accd", bufs=2) as accdp, \
             tc.tile_pool(name="yp", bufs=5) as yp, \
             tc.tile_pool(name="zp", bufs=2) as zp, \
             tc.tile_pool(name="smallp", bufs=8) as smallp, \
             tc.tile_pool(name="wmp", bufs=8) as wmp, \
             tc.tile_pool(name="psc", bufs=2, space="PSUM") as psc, \
             tc.tile_pool(name="psw", bufs=2, space="PSUM") as psw:

            # ---- constants (small vectors first so x loads start early;
            # the diagonal weight matrices are built on device: 1.18MB of
            # mostly-zero DMA replaced by an identity mask + 18 tiny muls) ----
            vv = singles.tile([P, 4, NCB], F32, tag="vv")
            nc.sync.dma_start(out=vv, in_=vecs.rearrange("v c k -> k v c"))
            wt = singles.tile([P, NCB, 9], F32, tag="wt")
            nc.sync.dma_start(out=wt, in_=wtap.rearrange("c t k -> k c t"))
            dg = singles.tile([P, NCB, 9, P], F32, tag="dg")
            ident = singles.tile([P, P], F32, tag="ident")
            from concourse.masks import make_identity
            make_identity(nc, ident)
            for _cb in range(NCB):
                for _ti in range(9):
                    nc.vector.tensor_scalar_mul(
                        dg[:, _cb, _ti, :], ident, wt[:, _cb, _ti : _ti + 1])
            pw = singles.tile([P, NCB, O], F32, tag="pw")
            nc.gpsimd.dma_start(out=pw, in_=pwt.rearrange("c k o -> k c o"))
            t2v = singles.tile([P, NOB], F32, tag="t2v")
            nc.gpsimd.dma_start(out=t2v, in_=t2d.rearrange("c k -> k c"))
            # vecs rows: 0=t1, 1=s2, 2=(t1-4), 3=-4
            scratch = singles.tile([P, H * W], F32, tag="scratch")

            units = [(b, cb) for b in range(B_PER) for cb in range(NCB)]
            xts = {}

            def emit_dma_and_init(u):
                b, cb = units[u]
                t1 = vv[:, 0, cb : cb + 1]
                w00 = wt[:, cb, 0:1]
                xt = xp.tile([P, PH, PW], F32, tag="xt")
                if u < 3:
                    # first use of this pool buffer: zero the pad strips
                    # (DMA only ever writes the interior, so they stay zero)
                    xf = xt.rearrange("p a b -> p (a b)")
                    nc.gpsimd.memset(xf[:, 0:PW], 0.0)
                    nc.gpsimd.memset(xf[:, (PH - 1) * PW : PH * PW], 0.0)
                    nc.gpsimd.memset(xt[:, 1 : PH - 1, 0:1], 0.0)
                    nc.gpsimd.memset(xt[:, 1 : PH - 1, PW - 1 : PW], 0.0)
                for ra, rb in ((0, 6), (6, 12), (12, 24), (24, 36),
                               (36, 46), (46, 56)):
                    nc.sync.dma_start(
                        out=xt[:, 1 + ra : 1 + rb, 1 : 1 + W],
                        in_=x[b, cb * P : (cb + 1) * P, ra:rb])
                d0 = NPE_LIST[u] * NCH
                rows = H - d0
                ad = accdp.tile([P, MAX_SPAN, W], F32, tag="ad")
                src = xt[:, 1 + d0 : 1 + H, 1 : 1 + W]
                # span init: acc = x*(s1*w00) + t1  (tap (0,0))
                nc.scalar.activation(
                    out=ad[:, :rows, :], in_=src,
                    func=mybir.ActivationFunctionType.Identity,
                    scale=w00, bias=t1)
                xts[u] = (xt, ad)

            def emit_conv(u):
                b, cb = units[u]
                xt, ad = xts[u]
                t1 = vv[:, 0, cb : cb + 1]
                t1m4 = vv[:, 2, cb : cb + 1]
                npe = NPE_LIST[u]
                d0 = npe * NCH
                rows = H - d0
                # yt is bf16 (fastest matmul dtype, z tolerance 2e-2 vs
                # ~5e-3 realized); the mask partials read the fp32 PSUM/acc
                # sources, so prune decisions stay exact
                yt = yp.tile([P, H, W], BF16, tag="yt")
                ysum = smallp.tile([P, npe + 1], F32, tag="ysum")
                xf = xt.rearrange("p a b -> p (a b)")

                # --- DVE span: 8 in-place 2D STT MACs ---
                for ti in range(1, 9):
                    dr, dc = TAPS[ti]
                    sc = wt[:, cb, ti : ti + 1]
                    nc.vector.scalar_tensor_tensor(
                        out=ad[:, :rows, :],
                        in0=xt[:, 1 + d0 + dr : 1 + H + dr, 1 + dc : 1 + W + dc],
                        scalar=sc, in1=ad[:, :rows, :],
                        op0=mybir.AluOpType.mult, op1=mybir.AluOpType.add)

                # --- PE chunks: 9 contiguous flat-window matmuls each ---
                for ci in range(npe):
                    r0 = ci * NCH
                    pt = psc.tile([P, NCH, PW], F32, tag="pt")
                    po = pt.rearrange("p a b -> p (a b)")
                    for ti, (dr, dc) in enumerate(TAPS):
                        s0 = (r0 + 1 + dr) * PW + 1 + dc
                        nc.tensor.matmul(
                            po[:, : NCH * PW], dg[:, cb, ti, :],
                            xf[:, s0 : s0 + NCH * PW],
                            start=(ti == 0), stop=(ti == 8))
                    nc.scalar.activation(
                        out=yt[:, r0 : r0 + NCH, :].rearrange("p h w -> p (h w)"),
                        in_=pt[:, :, 0:W],
                        func=mybir.ActivationFunctionType.Relu,
                        scale=1.0, bias=t1)
                    nc.scalar.activation(
                        out=scratch[:, : NCH * W], in_=pt[:, :, 0:W],
                        func=mybir.ActivationFunctionType.Relu,
                        scale=1.0, bias=t1m4,
                        accum_out=ysum[:, ci : ci + 1])

                # --- yt + mask partial from the span (acc already has t1) ---
                av = ad[:, :rows, :].rearrange("p h w -> p (h w)")
                nc.scalar.activation(
                    out=yt[:, d0:H, :].rearrange("p h w -> p (h w)"),
                    in_=av, func=mybir.ActivationFunctionType.Relu)
                nc.scalar.activation(
                    out=scratch[:, : rows * W], in_=av,
                    func=mybir.ActivationFunctionType.Relu,
                    bias=vv[:, 3, 0:1],
                    accum_out=ysum[:, npe : npe + 1])
                return yt, ysum

            def emit_mask(u, ysum):
                # plane max(y) > 4 <=> sum(relu(y-4)) > 0 (margin-checked)
                tot = smallp.tile([P, 1], F32, tag="tot")
                nc.vector.tensor_reduce(
                    out=tot, in_=ysum, axis=mybir.AxisListType.X,
                    op=mybir.AluOpType.add)
                m1 = smallp.tile([P, 1], F32, tag="m1")
                nc.vector.tensor_scalar(
                    out=m1, in0=tot, scalar1=0.0, scalar2=None,
                    op0=mybir.AluOpType.is_gt)
                return m1

            def emit_pw(b, ys, m1s):
                masks = []
                for cb in range(NCB):
                    wm = wmp.tile([P, O], BF16, tag="wm")
                    nc.vector.tensor_scalar_mul(wm, pw[:, cb, :], m1s[cb])
                    masks.append(wm)
                for ob in range(NOB):
                    s2 = vv[:, 1, ob : ob + 1]
                    t2 = t2v[:, ob : ob + 1]
                    zt = zp.tile([P, H, W], BF16, tag="zt")
                    for g0, gn in ((0, 3), (3, 3), (6, 1)):
                        pz = psw.tile([P, 3, 512], F32, tag="pz")
                        for kb in range(NCB):
                            for gi in range(gn):
                                r0 = (g0 + gi) * NCH
                                rhs = ys[kb][:, r0 : r0 + NCH, :].rearrange(
                                    "p h w -> p (h w)")
                                nc.tensor.matmul(
                                    pz[:, gi, : NCH * W],
                                    masks[kb][:, ob * P : (ob + 1) * P], rhs,
                                    start=(kb == 0), stop=(kb == NCB - 1))
                        nc.scalar.activation(
                            out=zt[:, g0 * NCH : (g0 + gn) * NCH, :].rearrange(
                                "p h w -> p (h w)"),
                            in_=pz[:, :gn, : NCH * W],
                            func=mybir.ActivationFunctionType.Relu,
                            scale=s2, bias=t2)
                    for za, zb in [(k * 7, k * 7 + 7) for k in range(8)]:
                        nc.gpsimd.dma_start(
                            out=zout[b, ob * P : (ob + 1) * P, za:zb],
                            in_=zt[:, za:zb, :])

            # ---- software-pipelined emission ----
            # mask ops are pended one unit (so the in-order DVE/ACT queues
            # don't stall on yt completion ahead of the next unit's work);
            # PW for batch b lands right after batch b+1's first conv unit
            yts = {}
            ysums = {}
            m1s = {}
            emit_dma_and_init(0)
            for u in range(len(units)):
                if u + 1 < len(units):
                    emit_dma_and_init(u + 1)
                yts[u], ysums[u] = emit_conv(u)
                if u >= 2:
                    pu = u - 2
                    m1s[pu] = emit_mask(pu, ysums[pu])
                    pb, pcb = units[pu]
                    if pcb == NCB - 1:
                        emit_pw(pb, [yts[pb * NCB + k] for k in range(NCB)],
                                [m1s[pb * NCB + k] for k in range(NCB)])
            for pu in (len(units) - 2, len(units) - 1):
                m1s[pu] = emit_mask(pu, ysums[pu])
                pb, pcb = units[pu]
                if pcb == NCB - 1:
                    emit_pw(pb, [yts[pb * NCB + k] for k in range(NCB)],
                            [m1s[pb * NCB + k] for k in range(NCB)])

    nc.compile()
    return nc


def kernel(x, dw_w, dw_b, bn1_gamma, bn1_beta, bn1_mean, bn1_var,
           pw_w, pw_b, bn2_gamma, bn2_beta, bn2_mean, bn2_var):
    # ---- host-side parameter folding (O(C) work only) ----
    s1 = (bn1_gamma / np.sqrt(bn1_var + EPS)).astype(np.float32)
    t1 = ((dw_b - bn1_mean) * s1 + bn1_beta).astype(np.float32)
    s2 = (bn2_gamma / np.sqrt(bn2_var + EPS)).astype(np.float32)
    t2 = ((pw_b - bn2_mean) * s2 + bn2_beta).astype(np.float32)

    # dw weights pre-scaled by s1: conv output is pre-relu y minus t1
    wfold = (dw_w[:, 0, :, :] * s1[:, None, None]).astype(np.float32)  # [C,3,3]
    wtap = np.zeros((NCB, 9, P), dtype=np.float32)
    for cb in range(NCB):
        for ti, (dr, dc) in enumerate(TAPS):
            wtap[cb, ti] = wfold[cb * P : (cb + 1) * P, dr + 1, dc + 1]

    pwt = np.ascontiguousarray(
        pw_w[:, :, 0, 0].T.reshape(NCB, P, O)).astype(np.float32)
    vecs = np.stack([t1.reshape(NCB, P), s2.reshape(NCB, P),
                     (t1 - DW_THRESH).reshape(NCB, P),
                     np.full((NCB, P), -DW_THRESH, np.float32)], axis=0)
    t2d = t2.reshape(NOB, P)

    nc = _build()

    in_maps = []
    for c in range(NCORES):
        in_maps.append({
            "x": np.ascontiguousarray(x[c * B_PER : (c + 1) * B_PER]),
            "wtap": wtap, "pwt": pwt,
            "vecs": np.ascontiguousarray(vecs), "t2d": np.ascontiguousarray(t2d),
        })
    if TRACE:
        _install_trace_hook()
    res = run_bass_kernel_spmd(nc, in_maps, core_ids=list(range(NCORES)),
                               trace=TRACE)
    global LAST_RESULTS
    LAST_RESULTS = res
    out = np.concatenate(
        [res.results[c]["z"].astype(np.float32) for c in range(NCORES)], axis=0)
    return out
